# revision 1
# baseline (speedup 1.0000x reference)
"""Trainium2 Bass kernel for a margin-softmax cross-entropy loss.

Reference computation (B=4096, D=512, C=10575):
    original = feats @ w                         # [B, C]
    means    = centers / counts[:, None]
    mn       = means / ||means||                 # unit rows
    dists    = mn[labels] @ mn.T                 # [B, C]
    logits   = original + onehot(labels) * dists # only label column changes
    loss     = mean(CE(logits, labels))

Key identities used:
  * Only the label column of `dists` survives the onehot mask, and
    dists[i, labels[i]] = |mn|^2 ~ 1.0 (host-computed from centers/counts).
  * logits are bounded (|logit| < ~3) so sum(exp) needs no max-shift;
    CE = log(sum_j exp(l_j)) - l_label.
  * Cross-entropy needs only two per-row scalars from the [B, C] logits:
    S_i = sum_j exp(l_ij) and t_i = l_i,label(i). t_i is recomputed on
    the host from the same fp8-quantized operands the device GEMM uses
    (f64 dot, ~1e-7 from the device's f32 PSUM value - far below what
    the S correction needs), so no device gather/diag GEMM is needed.

Device work per core (classes sharded 8 ways, 1328 padded cols): one
[4096 x 512] @ [512 x 1328] fp8 DoubleRow GEMM. The exp+row-sum of each
128-row strip is split across two engines so neither outpaces the PE:
  * 29 strips on ScalarE: one full-width ACTIVATE Exp (the free affine
    scale undoes the fp8 pre-scale) into f16, then VectorE sums via a
    3-stage tree (two bf16/f16 half-adds at 2x mode + one short reduce
    - ~1.0us vs ~1.4us for a flat reduce).
  * 3 strips entirely on VectorE via Schraudolph fast-exp:
    tensor_scalar computes round(l*2^7/ln2 + M0) into int16 (= the bit
    pattern of bf16(exp(l)), ~2% per-element sawtooth that averages out
    over 10575 classes), then the same tree sums the int16 buffer
    bitcast as bf16. M0 is tuned to make the estimator unbiased under
    the logit distribution; the host replicates the integer formula for
    the label/pad corrections.

Startup: the critical pack [wS_k0|fT0_k0 | wS_k1|fT0_k1] rides two
chained DMAs (first matmuls only need the k0 half), then the gated fT
remainder. A DVE memset + 5 dummy matmuls warm the PE HAM clock gate
during the DMA wait so real matmuls start at 2.4 GHz. The last strip's
exp+reduce is split so the post-GEMM tail is short, and the final
output DMA is issued from the Vector engine right after the last
reduce (same-engine chaining, no cross-engine semaphore hop).

Host combines the 8 partial sums and applies the margin correction:
    S' = S - E(t) + exp(t + d);  nll = log(S') - (t + d)
where E(t) is the device's own label-column contribution (spline exp or
fast-exp depending on the strip). Zero-padded w columns contribute
exp(0)=1 (ACT strips) or fastexp(0) (DVE strips); the host subtracts
the pad count times the per-strip constant.
"""

from contextlib import ExitStack

import ml_dtypes
import numpy as np

import concourse.bass as bass
import concourse.tile as tile
from concourse import bacc, mybir
from concourse.bass_utils import run_bass_kernel_spmd

B = 4096
D = 512
C = 10575
NCORES = 8
CS_BASE = 1322        # real classes on cores 0..6; core 7 gets 1321
CSH = 1328            # padded per-core class count
CW = (512, 512, 304)  # class-tile widths (PSUM bank-aligned starts)
CO = (0, 512, 1024)   # class-tile offsets
KP = 2                # fp8 DoubleRow k-pairs (256 contraction each)
BT = B // 128         # 32 batch tiles
BSH = B // NCORES     # 512 rows of label logits per core
WSCALE = 64.0         # fp8 pre-scale for w (subnormal-range fix), undone in exp

# Schraudolph fast-exp: bf16 bits of exp(l) ~ round(l * 2^7/ln2 + M0).
# M0 tuned for zero mean bias of sum(exp) under the logit distribution
# N(0, 0.304) (see calib.py); K1S folds in the 1/WSCALE PSUM pre-scale.
K1 = 184.6650292
M0 = 16248.7173
K1S = float(np.float32(K1 / WSCALE))
# strips whose exp+sum runs on VectorE via fast-exp (rest on ScalarE).
# Empty: the ACT exp keeps pace with the PE, and the fast-exp's longer
# PSUM hold (1540ns tensor_scalar vs 1370ns ACTIVATE) costs more than
# the ACT relief buys.
DVE_STRIPS = ()
HH = CSH // 2         # 664: half width for reduce-tree stage 1
QQ = HH // 2          # 332: quarter width for stage 2

BF16 = mybir.dt.bfloat16
F16 = mybir.dt.float16
FP8 = mybir.dt.float8e4
F32 = mybir.dt.float32
I16 = mybir.dt.int16

_CACHE = {}


def _build_nc():
    nc = bacc.Bacc("TRN2", debug=False, target_bir_lowering=False)

    # critical pack: [wS_k0|fT0_k0 | wS_k1|fT0_k1] per partition
    HK = 2 * CSH + 1024
    H1 = KP * HK
    head1 = nc.dram_tensor("head1", [128, H1], FP8, kind="ExternalInput").ap()
    fTr = nc.dram_tensor("fTr", [KP, 128, 2, B - 512], FP8, kind="ExternalInput").ap()
    outS = nc.dram_tensor("outS", [128, BT + 1], F32, kind="ExternalOutput").ap()

    with tile.TileContext(nc) as tc, ExitStack() as ctx:
        consts = ctx.enter_context(tc.tile_pool(name="consts", bufs=1))
        psums = ctx.enter_context(tc.tile_pool(name="psums", bufs=2, space="PSUM"))
        psumw = ctx.enter_context(tc.tile_pool(name="psumw", bufs=1, space="PSUM"))
        epool = ctx.enter_context(tc.tile_pool(name="epool", bufs=3))
        hpool = ctx.enter_context(tc.tile_pool(name="hpool", bufs=2))
        ypool = ctx.enter_context(tc.tile_pool(name="ypool", bufs=2))
        outs = ctx.enter_context(tc.tile_pool(name="outs", bufs=1))

        # PE HAM warm-up: dummy matmuls during the DMA head so real matmuls
        # start at 2.4 GHz. The HAM busy-window needs ~3.4us of SUSTAINED
        # activity with no idle gap before the real stream, so the warmup is
        # sized to bridge until the head DMA lands (~12us). Memset on GpSimd
        # (the earliest engine to dispatch after instruction load).
        warm = consts.tile([128, 384], BF16, tag="warm")
        nc.gpsimd.memset(warm[:], 0.0)
        pwt = psumw.tile([128, 384], F32, tag="pw")
        for _ in range(15):
            nc.tensor.matmul(out=pwt[:], lhsT=warm[:, 0:128],
                             rhs=warm[:], start=True, stop=True)
        # prime the ScalarE exp table (~2.7us ACT_TABLE_LOAD) during the DMA
        # wait so the first real exp doesn't pay it
        eprime = consts.tile([128, 1], F16, tag="eprime")
        nc.scalar.activation(
            out=eprime[:], in_=warm[:, 0:1].bitcast(F16),
            func=mybir.ActivationFunctionType.Exp, scale=1.0,
        )

        # DMAs in strict priority order on one ring: one monolithic head
        # (everything strips 0-3 need) -> fT remainder in two chunks. Extra
        # chunking costs more than it buys: each chained DMA adds ~1.3us of
        # descriptor-generation + completion-semaphore latency.
        head1_sb = consts.tile([128, H1], FP8, tag="head1")
        h1_dma = nc.gpsimd.dma_start(out=head1_sb[:], in_=head1[:])
        # per-(k, class-tile) SBUF views of the stationary pack; layout:
        #   [wS_k0_c0|fT0_k0 | wS_k0_c1|wS_k0_c2 | wS_k1_c*|fT0_k1]
        CUT_A = 2 * 512 + 1024            # wS_k0_c0 + fT0_k0
        CUT_B = CUT_A + 2 * (CSH - 512)   # + wS_k0_c1, wS_k0_c2
        OFF_K0 = {0: 0, 1: CUT_A, 2: CUT_A + 2 * 512}
        OFF_K1 = {0: CUT_B, 1: CUT_B + 2 * 512, 2: CUT_B + 2 * 1024}
        wS_sb = [
            {c: head1_sb[:, off:off + 2 * CW[c]].rearrange(
                "p (i n) -> p i n", i=2)
             for c, off in offs.items()}
            for offs in (OFF_K0, OFF_K1)
        ]
        FT1_OFF = CUT_B + 2 * CSH
        fT0_sb = [
            head1_sb[:, 2 * 512:CUT_A].rearrange("p (i n) -> p i n", i=2),
            head1_sb[:, FT1_OFF:FT1_OFF + 1024].rearrange(
                "p (i n) -> p i n", i=2),
        ]
        fTr_sb = []
        for k in range(KP):
            t = consts.tile([128, 2, B - 512], FP8, tag=f"fTr{k}")
            fTr_sb.append(t)
        a_dmas = []
        for k in range(KP):
            d = nc.sync.dma_start(
                out=fTr_sb[k][:, :, 0:1536], in_=fTr[k][:, :, 0:1536])
            tile.add_dep_helper(d.ins, h1_dma.ins, reason="fTr-a after head1")
            a_dmas.append(d)
        for k in range(KP):
            d = nc.sync.dma_start(
                out=fTr_sb[k][:, :, 1536:B - 512], in_=fTr[k][:, :, 1536:B - 512])
            for ad in a_dmas:
                tile.add_dep_helper(d.ins, ad.ins, reason="fTr-b after fTr-a")

        # main GEMM; per-strip exp+row-sum split across ScalarE / VectorE
        st = outs.tile([128, BT + 1], F32, tag="st")

        def tree_sum(src_lo, src_hi, width, dst, dtype):
            """2-stage half-add tree + short reduce: sum(src)/partition."""
            h1 = hpool.tile([128, width], dtype, tag="h1")
            nc.vector.tensor_add(out=h1[:], in0=src_lo, in1=src_hi)
            q = width // 2
            h2 = hpool.tile([128, q], dtype, tag="h2")
            nc.vector.tensor_add(out=h2[:], in0=h1[:, 0:q], in1=h1[:, q:width])
            nc.vector.tensor_reduce(
                out=dst, in_=h2[:],
                axis=mybir.AxisListType.X, op=mybir.AluOpType.add,
            )

        for b in range(BT):
            ps = psums.tile([128, CSH], F32, tag="ps")
            for k in range(KP):
                for c in range(len(CW)):
                    lhsT = (fT0_sb[k][:, :, b * 128:(b + 1) * 128]
                            if b < 4 else
                            fTr_sb[k][:, :, (b - 4) * 128:(b - 3) * 128])
                    nc.tensor.matmul(
                        out=ps[:, CO[c]:CO[c] + CW[c]],
                        lhsT=lhsT,
                        rhs=wS_sb[k][c][:],
                        start=(k == 0),
                        stop=(k == KP - 1),
                        perf_mode=mybir.MatmulPerfMode.DoubleRow,
                    )
            if b in DVE_STRIPS:
                # VectorE fast-exp: int16 bf16-bit pattern, then tree-sum
                y = ypool.tile([128, CSH], I16, tag="y")
                nc.vector.tensor_scalar(
                    out=y[:], in0=ps[:], scalar1=K1S, scalar2=float(M0),
                    op0=mybir.AluOpType.mult, op1=mybir.AluOpType.add,
                )
                tree_sum(y[:, 0:HH].bitcast(BF16), y[:, HH:CSH].bitcast(BF16),
                         HH, st[:, b:b + 1], BF16)
            elif b == BT - 1:
                # last strip: split exp with accum_out on ScalarE - the
                # accumulator surcharge (~280ns) is cheaper than queueing
                # behind VectorE's pending trees, and the final DMA chains
                # on the same (Activation) ring
                e = epool.tile([128, CSH], F16, tag="e")
                nc.scalar.activation(
                    out=e[:, 0:1024], in_=ps[:, 0:1024],
                    func=mybir.ActivationFunctionType.Exp,
                    scale=float(1.0 / WSCALE),
                    accum_out=st[:, b:b + 1],
                )
                nc.scalar.activation(
                    out=e[:, 1024:CSH], in_=ps[:, 1024:CSH],
                    func=mybir.ActivationFunctionType.Exp,
                    scale=float(1.0 / WSCALE),
                    accum_out=st[:, BT:BT + 1],
                )
            else:
                e = epool.tile([128, CSH], F16, tag="e")
                nc.scalar.activation(
                    out=e[:],
                    in_=ps[:],
                    func=mybir.ActivationFunctionType.Exp,
                    scale=float(1.0 / WSCALE),
                )
                tree_sum(e[:, 0:HH], e[:, HH:CSH], HH, st[:, b:b + 1], F16)
            if b == 15:
                nc.sync.dma_start(out=outS[:, 0:16], in_=st[:, 0:16])
            if b == BT - 4:
                nc.sync.dma_start(
                    out=outS[:, 16:BT - 4], in_=st[:, 16:BT - 4])
        # final chunk from the Activation ring (HWDGE, shorter setup than
        # the sync ring's SWDGE path)
        nc.scalar.dma_start(out=outS[:, BT - 4:], in_=st[:, BT - 4:])

    nc.compile()
    return nc


def _core_sizes():
    sizes = [CS_BASE] * (NCORES - 1) + [C - CS_BASE * (NCORES - 1)]
    starts = np.concatenate([[0], np.cumsum(sizes)[:-1]]).astype(np.int64)
    return np.array(sizes, dtype=np.int64), starts


def _prepare_inputs(feats, labels, w):
    sizes, starts = _core_sizes()

    in_maps = []
    fp8_feats = feats.astype(ml_dtypes.float8_e4m3)
    for p in range(NCORES):
        # roll this core's own rows to the front (SPMD: same program on all
        # cores; strip b covers rolled rows b*128..b*128+127)
        frolled = np.roll(fp8_feats, -p * BSH, axis=0)
        # fp8 DoubleRow layout: element [kp, q, i, b] = frolled[b, kp*256 + i*128 + q]
        fT_host = np.ascontiguousarray(
            frolled.reshape(B, KP, 2, 128).transpose(1, 3, 2, 0)
        )
        fTr_host = np.ascontiguousarray(fT_host[:, :, :, 512:])
        c0, sz = int(starts[p]), int(sizes[p])
        wp = np.zeros((D, CSH), dtype=np.float32)
        wp[:, :sz] = w[:, c0:c0 + sz] * WSCALE
        wS_host = np.ascontiguousarray(
            wp.reshape(KP, 2, 128, CSH).transpose(0, 2, 1, 3)
        ).astype(ml_dtypes.float8_e4m3)

        # pack order: wS_k0_c0 | fT0_k0 | wS_k0_c1 | wS_k0_c2
        #           | wS_k1_c0 | wS_k1_c1 | wS_k1_c2 | fT0_k1
        # (each class tile's [2, n] interleave kept contiguous so the device
        #  per-ctile rearrange views line up)
        def ctile(k, c):
            return np.ascontiguousarray(
                wS_host[k][:, :, CO[c]:CO[c] + CW[c]]).reshape(128, -1)

        head1_host = np.concatenate(
            [ctile(0, 0),
             np.ascontiguousarray(fT_host[0][:, :, 0:512]).reshape(128, -1),
             ctile(0, 1), ctile(0, 2),
             ctile(1, 0), ctile(1, 1), ctile(1, 2),
             np.ascontiguousarray(fT_host[1][:, :, 0:512]).reshape(128, -1)],
            axis=1,
        )
        in_maps.append({
            "head1": np.ascontiguousarray(head1_host),
            "fTr": fTr_host,
        })
    return in_maps


def _run(in_maps, trace=False):
    if "nc" not in _CACHE:
        _CACHE["nc"] = _build_nc()
    nc = _CACHE["nc"]
    return run_bass_kernel_spmd(
        nc, in_maps, core_ids=list(range(NCORES)), trace=trace
    )


def _fastexp_host(ps_vals):
    """Replica of the device fast-exp for f32 PSUM values:
    bf16 bits = rint(f32(f32(ps * K1S) + M0)), read back as bf16 floats."""
    x = np.asarray(ps_vals, dtype=np.float32)
    y = np.float32(x * np.float32(K1S)) + np.float32(M0)
    return np.rint(y).astype(np.int16).view(ml_dtypes.bfloat16).astype(np.float64)


def kernel(feats, labels, centers, counts, w, _trace=False, _ret_res=False):
    feats = np.asarray(feats, dtype=np.float32)
    labels_i = np.asarray(labels).astype(np.int64)
    centers = np.asarray(centers, dtype=np.float32)
    counts = np.asarray(counts, dtype=np.float32)
    w = np.asarray(w, dtype=np.float32)

    in_maps = _prepare_inputs(feats, labels_i, w)
    res = _run(in_maps, trace=_trace)

    sizes, starts = _core_sizes()

    # margin d_c = |means_c / ||means_c|| |^2 (~1.0), matching the reference's
    # f32 normalize-then-dot on the label diagonal
    means = (centers / counts[:, None]).astype(np.float32)
    nrm = np.sqrt((means.astype(np.float32) ** 2).sum(axis=1, keepdims=True))
    mn = (means / nrm).astype(np.float32)
    dsq = (mn.astype(np.float64) ** 2).sum(axis=1)       # [C]
    d = dsq[labels_i]                                    # [B]

    # label-column PSUM value, recomputed on host from the same fp8 operands
    # the device GEMM consumed (f64 dot ~ the device's f32 tree sum)
    f8 = feats.astype(ml_dtypes.float8_e4m3).astype(np.float64)      # [B, D]
    w8 = (w * WSCALE).astype(ml_dtypes.float8_e4m3).astype(np.float64)
    t_ps = np.einsum("bd,bd->b", f8, w8[:, labels_i].T)              # [B]
    t = t_ps / WSCALE

    # per-strip pad constant: exp(0)=1 on ACT strips, fastexp(0) on DVE
    fastexp0 = float(_fastexp_host(np.zeros(1))[0])
    is_dve = np.zeros(BT, dtype=bool)
    is_dve[list(DVE_STRIPS)] = True
    padval = np.where(is_dve, fastexp0, 1.0)             # [BT]

    S_tot = np.zeros(B, dtype=np.float64)
    for p in range(NCORES):
        # outS[q, b] is rolled row b*128 + q = original row (b*128+q+p*BSH)%B
        sp = res.results[p]["outS"].astype(np.float64)   # [128, BT+1]
        sp[:, BT - 1] += sp[:, BT]                       # merge split last strip
        S_p = sp[:, :BT].T.reshape(B)                    # rolled rows
        pad_p = float(CSH - sizes[p])
        S_p = S_p - pad_p * np.repeat(padval, 128)
        S_tot += np.roll(S_p, p * BSH)

    # subtract the device's own label-column contribution: row i's label
    # class lives in shard p*, where row i sits in strip b* (rolled)
    p_star = np.minimum(labels_i // CS_BASE, NCORES - 1)
    b_star = ((np.arange(B) - p_star * BSH) % B) // 128
    lab_dev = np.where(
        is_dve[b_star],
        _fastexp_host(t_ps),
        np.exp(t),
    )
    z = S_tot - lab_dev + np.exp(t + d)
    nll = np.log(z) - (t + d)
    loss = np.float32(nll.mean())
    out = np.array(loss, dtype=np.float32)
    if _ret_res:
        return out, res
    return out



# revision 6
# speedup vs baseline: 1.0028x; 1.0028x over previous
"""Trainium2 Bass kernel for a margin-softmax cross-entropy loss.

Baseline (62882ns): see git-less problem dir. Restored from session read.
"""

from contextlib import ExitStack

import ml_dtypes
import numpy as np

import concourse.bass as bass
import concourse.tile as tile
from concourse import bacc, mybir
from concourse.bass_utils import run_bass_kernel_spmd

B = 4096
D = 512
C = 10575
NCORES = 8
CS_BASE = 1322        # real classes on cores 0..6; core 7 gets 1321
CSH = 1328            # padded per-core class count
CW = (512, 512, 304)  # class-tile widths (PSUM bank-aligned starts)
CO = (0, 512, 1024)   # class-tile offsets
KP = 2                # fp8 DoubleRow k-pairs (256 contraction each)
BT = B // 128         # 32 batch tiles
BSH = B // NCORES     # 512 rows of label logits per core
WSCALE = 64.0         # fp8 pre-scale for w (subnormal-range fix), undone in exp

# Schraudolph fast-exp: bf16 bits of exp(l) ~ round(l * 2^7/ln2 + M0).
K1 = 184.6650292
M0 = 16248.7173
K1S = float(np.float32(K1 / WSCALE))
DVE_STRIPS = ()
HH = CSH // 2         # 664: half width for reduce-tree stage 1
QQ = HH // 2          # 332: quarter width for stage 2

BF16 = mybir.dt.bfloat16
F16 = mybir.dt.float16
FP8 = mybir.dt.float8e4
F32 = mybir.dt.float32
I16 = mybir.dt.int16

_CACHE = {}


def _build_nc():
    nc = bacc.Bacc("TRN2", debug=False, target_bir_lowering=False)

    # critical pack: [wS_k0|fT0_k0 | wS_k1|fT0_k1] per partition
    HK = 2 * CSH + 1024
    H1 = KP * HK
    head1 = nc.dram_tensor("head1", [128, H1], FP8, kind="ExternalInput").ap()
    fTr = nc.dram_tensor("fTr", [KP, 128, 2, B - 512], FP8, kind="ExternalInput").ap()
    outS = nc.dram_tensor("outS", [128, BT + 1], F32, kind="ExternalOutput").ap()

    with tile.TileContext(nc) as tc, ExitStack() as ctx:
        consts = ctx.enter_context(tc.tile_pool(name="consts", bufs=1))
        psums = ctx.enter_context(tc.tile_pool(name="psums", bufs=2, space="PSUM"))
        psumw = ctx.enter_context(tc.tile_pool(name="psumw", bufs=1, space="PSUM"))
        epool = ctx.enter_context(tc.tile_pool(name="epool", bufs=3))
        hpool = ctx.enter_context(tc.tile_pool(name="hpool", bufs=2))
        ypool = ctx.enter_context(tc.tile_pool(name="ypool", bufs=2))
        outs = ctx.enter_context(tc.tile_pool(name="outs", bufs=1))

        # input DMAs first, chained on the one SP HWDGE ring in priority
        # order: the queue drains in issue order, so head-A (everything
        # strip-0's k0 matmuls need) hits the wire first.
        CUT_A = 2 * 512 + 1024            # wS_k0_c0 + fT0_k0
        CUT_B = CUT_A + 2 * (CSH - 512)   # + wS_k0_c1, wS_k0_c2 (k0 half end)
        OFF_K0 = {0: 0, 1: CUT_A, 2: CUT_A + 2 * 512}
        OFF_K1 = {0: CUT_B, 1: CUT_B + 2 * 512, 2: CUT_B + 2 * 1024}
        FT1_OFF = CUT_B + 2 * CSH
        FSPLIT = 1024                     # fTr chunk split (strips 4-11 | 12-31)

        head1_sb = consts.tile([128, H1], FP8, tag="head1")
        nc.sync.dma_start(out=head1_sb[:, 0:CUT_B], in_=head1[:, 0:CUT_B])
        nc.sync.dma_start(out=head1_sb[:, CUT_B:H1], in_=head1[:, CUT_B:H1])
        fTr_sb = []
        for k in range(KP):
            t = consts.tile([128, 2, B - 512], FP8, tag=f"fTr{k}")
            fTr_sb.append(t)
        for k in range(KP):
            nc.sync.dma_start(
                out=fTr_sb[k][:, :, 0:FSPLIT], in_=fTr[k][:, :, 0:FSPLIT])
        for k in range(KP):
            nc.sync.dma_start(
                out=fTr_sb[k][:, :, FSPLIT:B - 512],
                in_=fTr[k][:, :, FSPLIT:B - 512])

        # PE HAM warm-up
        warm = consts.tile([128, 384], BF16, tag="warm")
        nc.gpsimd.memset(warm[:], 0.0)
        pwt = psumw.tile([128, 384], F32, tag="pw")
        for _ in range(12):
            nc.tensor.matmul(out=pwt[:], lhsT=warm[:, 0:128],
                             rhs=warm[:], start=True, stop=True)
        eprime = consts.tile([128, 1], F16, tag="eprime")
        nc.scalar.activation(
            out=eprime[:], in_=warm[:, 0:1].bitcast(F16),
            func=mybir.ActivationFunctionType.Exp, scale=1.0,
        )

        wS_sb = [
            {c: head1_sb[:, off:off + 2 * CW[c]].rearrange(
                "p (i n) -> p i n", i=2)
             for c, off in offs.items()}
            for offs in (OFF_K0, OFF_K1)
        ]
        fT0_sb = [
            head1_sb[:, 2 * 512:CUT_A].rearrange("p (i n) -> p i n", i=2),
            head1_sb[:, FT1_OFF:FT1_OFF + 1024].rearrange(
                "p (i n) -> p i n", i=2),
        ]

        # main GEMM; per-strip exp+row-sum split across ScalarE / VectorE
        st = outs.tile([128, BT + 1], F32, tag="st")

        def tree_sum(src_lo, src_hi, width, dst, dtype):
            """2-stage half-add tree + short reduce: sum(src)/partition."""
            h1 = hpool.tile([128, width], dtype, tag="h1")
            nc.vector.tensor_add(out=h1[:], in0=src_lo, in1=src_hi)
            q = width // 2
            h2 = hpool.tile([128, q], dtype, tag="h2")
            nc.vector.tensor_add(out=h2[:], in0=h1[:, 0:q], in1=h1[:, q:width])
            nc.vector.tensor_reduce(
                out=dst, in_=h2[:],
                axis=mybir.AxisListType.X, op=mybir.AluOpType.add,
            )

        for b in range(BT):
            ps = psums.tile([128, CSH], F32, tag="ps")
            for k in range(KP):
                for c in range(len(CW)):
                    lhsT = (fT0_sb[k][:, :, b * 128:(b + 1) * 128]
                            if b < 4 else
                            fTr_sb[k][:, :, (b - 4) * 128:(b - 3) * 128])
                    nc.tensor.matmul(
                        out=ps[:, CO[c]:CO[c] + CW[c]],
                        lhsT=lhsT,
                        rhs=wS_sb[k][c][:],
                        start=(k == 0),
                        stop=(k == KP - 1),
                        perf_mode=mybir.MatmulPerfMode.DoubleRow,
                    )
            if b in DVE_STRIPS:
                y = ypool.tile([128, CSH], I16, tag="y")
                nc.vector.tensor_scalar(
                    out=y[:], in0=ps[:], scalar1=K1S, scalar2=float(M0),
                    op0=mybir.AluOpType.mult, op1=mybir.AluOpType.add,
                )
                tree_sum(y[:, 0:HH].bitcast(BF16), y[:, HH:CSH].bitcast(BF16),
                         HH, st[:, b:b + 1], BF16)
            elif b == BT - 1:
                e = epool.tile([128, CSH], F16, tag="e")
                nc.scalar.activation(
                    out=e[:, 0:1024], in_=ps[:, 0:1024],
                    func=mybir.ActivationFunctionType.Exp,
                    scale=float(1.0 / WSCALE),
                    accum_out=st[:, b:b + 1],
                )
                nc.scalar.activation(
                    out=e[:, 1024:CSH], in_=ps[:, 1024:CSH],
                    func=mybir.ActivationFunctionType.Exp,
                    scale=float(1.0 / WSCALE),
                    accum_out=st[:, BT:BT + 1],
                )
            else:
                e = epool.tile([128, CSH], F16, tag="e")
                nc.scalar.activation(
                    out=e[:],
                    in_=ps[:],
                    func=mybir.ActivationFunctionType.Exp,
                    scale=float(1.0 / WSCALE),
                )
                hh = hpool.tile([128, HH], F16, tag="h1")
                nc.vector.scalar_tensor_tensor(
                    out=hh[:], in0=e[:, 0:HH], scalar=1.0, in1=e[:, HH:CSH],
                    op0=mybir.AluOpType.mult, op1=mybir.AluOpType.add,
                    accum_out=st[:, b:b + 1],
                )
            if b == 15:
                nc.sync.dma_start(out=outS[:, 0:16], in_=st[:, 0:16])
            if b == BT - 4:
                nc.sync.dma_start(
                    out=outS[:, 16:BT - 4], in_=st[:, 16:BT - 4])
        # final chunk from the Activation ring
        nc.scalar.dma_start(out=outS[:, BT - 4:], in_=st[:, BT - 4:])

    nc.compile()
    return nc


def _core_sizes():
    sizes = [CS_BASE] * (NCORES - 1) + [C - CS_BASE * (NCORES - 1)]
    starts = np.concatenate([[0], np.cumsum(sizes)[:-1]]).astype(np.int64)
    return np.array(sizes, dtype=np.int64), starts


def _prepare_inputs(feats, labels, w):
    sizes, starts = _core_sizes()

    in_maps = []
    fp8_feats = feats.astype(ml_dtypes.float8_e4m3)
    for p in range(NCORES):
        frolled = np.roll(fp8_feats, -p * BSH, axis=0)
        fT_host = np.ascontiguousarray(
            frolled.reshape(B, KP, 2, 128).transpose(1, 3, 2, 0)
        )
        fTr_host = np.ascontiguousarray(fT_host[:, :, :, 512:])
        c0, sz = int(starts[p]), int(sizes[p])
        wp = np.zeros((D, CSH), dtype=np.float32)
        wp[:, :sz] = w[:, c0:c0 + sz] * WSCALE
        wS_host = np.ascontiguousarray(
            wp.reshape(KP, 2, 128, CSH).transpose(0, 2, 1, 3)
        ).astype(ml_dtypes.float8_e4m3)

        def ctile(k, c):
            return np.ascontiguousarray(
                wS_host[k][:, :, CO[c]:CO[c] + CW[c]]).reshape(128, -1)

        head1_host = np.concatenate(
            [ctile(0, 0),
             np.ascontiguousarray(fT_host[0][:, :, 0:512]).reshape(128, -1),
             ctile(0, 1), ctile(0, 2),
             ctile(1, 0), ctile(1, 1), ctile(1, 2),
             np.ascontiguousarray(fT_host[1][:, :, 0:512]).reshape(128, -1)],
            axis=1,
        )
        in_maps.append({
            "head1": np.ascontiguousarray(head1_host),
            "fTr": fTr_host,
        })
    return in_maps


def _run(in_maps, trace=False):
    if "nc" not in _CACHE:
        _CACHE["nc"] = _build_nc()
    nc = _CACHE["nc"]
    return run_bass_kernel_spmd(
        nc, in_maps, core_ids=list(range(NCORES)), trace=trace
    )


def _fastexp_host(ps_vals):
    x = np.asarray(ps_vals, dtype=np.float32)
    y = np.float32(x * np.float32(K1S)) + np.float32(M0)
    return np.rint(y).astype(np.int16).view(ml_dtypes.bfloat16).astype(np.float64)


def kernel(feats, labels, centers, counts, w, _trace=False, _ret_res=False):
    feats = np.asarray(feats, dtype=np.float32)
    labels_i = np.asarray(labels).astype(np.int64)
    centers = np.asarray(centers, dtype=np.float32)
    counts = np.asarray(counts, dtype=np.float32)
    w = np.asarray(w, dtype=np.float32)

    in_maps = _prepare_inputs(feats, labels_i, w)
    res = _run(in_maps, trace=_trace)

    sizes, starts = _core_sizes()

    means = (centers / counts[:, None]).astype(np.float32)
    nrm = np.sqrt((means.astype(np.float32) ** 2).sum(axis=1, keepdims=True))
    mn = (means / nrm).astype(np.float32)
    dsq = (mn.astype(np.float64) ** 2).sum(axis=1)       # [C]
    d = dsq[labels_i]                                    # [B]

    f8 = feats.astype(ml_dtypes.float8_e4m3).astype(np.float64)      # [B, D]
    w8 = (w * WSCALE).astype(ml_dtypes.float8_e4m3).astype(np.float64)
    t_ps = np.einsum("bd,bd->b", f8, w8[:, labels_i].T)              # [B]
    t = t_ps / WSCALE

    fastexp0 = float(_fastexp_host(np.zeros(1))[0])
    is_dve = np.zeros(BT, dtype=bool)
    is_dve[list(DVE_STRIPS)] = True
    padval = np.where(is_dve, fastexp0, 1.0)             # [BT]

    S_tot = np.zeros(B, dtype=np.float64)
    for p in range(NCORES):
        sp = res.results[p]["outS"].astype(np.float64)   # [128, BT+1]
        sp[:, BT - 1] += sp[:, BT]                       # merge split last strip
        S_p = sp[:, :BT].T.reshape(B)                    # rolled rows
        pad_p = float(CSH - sizes[p])
        S_p = S_p - pad_p * np.repeat(padval, 128)
        S_tot += np.roll(S_p, p * BSH)

    p_star = np.minimum(labels_i // CS_BASE, NCORES - 1)
    b_star = ((np.arange(B) - p_star * BSH) % B) // 128
    lab_dev = np.where(
        is_dve[b_star],
        _fastexp_host(t_ps),
        np.exp(t),
    )
    z = S_tot - lab_dev + np.exp(t + d)
    nll = np.log(z) - (t + d)
    loss = np.float32(nll.mean())
    out = np.array(loss, dtype=np.float32)
    if _ret_res:
        return out, res
    return out
